# revision 20
# baseline (speedup 1.0000x reference)
"""CopyGenerator kernel for Trainium2 (Bass/Tile), batch-parallel over 8 cores.

Core c owns batch c end-to-end: attention, full-vocab generation scores,
softmax normalizer Z, copy-distribution scatter, and the blended log output.
No collectives — each core's NEFF is fully independent, so per-core exec
time is its own busy span regardless of dispatch skew across cores.

Host-side prep (numpy, once per call):
  emb8  = (emb.T * 32) cast to fp8e4, [KC/2, 2, P, V]  (DoubleRow k-pairs)
  hh8   = htgt[:,c,:].T cast fp8e4 for the gen matmul
  hhT   = htgt/hsrc own-batch transposes fp16 (attention)
  qwT   = q_w.T fp16;  w2 = f_w.T @ copy_w.T, b2 = copy_w @ f_b + copy_b

Device pipeline per core (vocab in 2000-col blocks; first two half-width
so the PE starts sooner; attention emitted after the first gen block):
  pass 1: DMA emb8 block -> gen matmul (PE fp8 DoubleRow: 2 instrs per
          500-col x 512-deep) -> exp(score/32) over 1000 cols (ACT, fused
          row-sum) -> e[:, block] SBUF-resident ([128, 32000] f16)
  Z = rowsum; c1 = (1-a)/Z, c2 = a*Z/(1-a)
  pass 2: onehot(src) per 2000 cols (DVE is_equal vs f16 iota, 2x mode) ->
          copy matmul (PE fp16, 500-col) -> blend c2*cp+e (DVE) ->
          Ln with scale c1 over 2000 cols (ACT) -> DMA out rows
"""

import sys

sys.path.insert(0, "/opt/trn_rl_repo")

import numpy as np

from concourse import bass, bacc, mybir
import concourse.tile as tile
from concourse.bass_utils import run_bass_kernel_spmd
from concourse.masks import make_identity

NT, NS, BS, D, V = 128, 128, 8, 512, 32000
NCORES = 8
P = 128
KC = D // P  # 4 contraction chunks of 128
G = KC // 2  # 2 DoubleRow pair-groups (256-deep each)
CH = 500  # cols per PSUM bank (f32)
WCH = 2 * CH  # cols per exp activate (2 banks)
DCH = 2000  # cols per embT DMA load / onehot / Ln / out store
NDMA = V // DCH  # 16
NZ = V // WCH  # 32 partial-Z columns
ESCALE = 32.0  # host scales emb by 32 into fp8e4 normal range; exp undoes
F32 = mybir.dt.float32
F16 = mybir.dt.float16
F8 = mybir.dt.float8e4
I16 = mybir.dt.int16
AF = mybir.ActivationFunctionType
ALU = mybir.AluOpType
DR = mybir.MatmulPerfMode.DoubleRow
INV_SQRT_D = 1.0 / float(np.sqrt(np.float32(D)))


def build_kernel():
    nc = bacc.Bacc(
        "TRN2",
        target_bir_lowering=False,
        debug=False,
        enable_asserts=False,
        num_devices=NCORES,
    )
    emb8 = nc.dram_tensor("emb8", [G, 2, P, V], F8, kind="ExternalInput").ap()
    hh8 = nc.dram_tensor("hh8", [G, 2, P, NT], F8, kind="ExternalInput").ap()
    hhT = nc.dram_tensor("hhT", [KC, P, 2, P], F16, kind="ExternalInput").ap()
    qwT = nc.dram_tensor("qwT", [KC, P, D], F16, kind="ExternalInput").ap()
    qb = nc.dram_tensor("qb", [1, D], F16, kind="ExternalInput").ap()
    w2 = nc.dram_tensor("w2", [KC, P], F32, kind="ExternalInput").ap()
    b2 = nc.dram_tensor("b2", [1, 1], F32, kind="ExternalInput").ap()
    src = nc.dram_tensor("src", [NS, 1], F32, kind="ExternalInput").ap()
    out = nc.dram_tensor("out", [NT, V], F32, kind="ExternalOutput").ap()

    with tile.TileContext(nc) as tc:
        _emit(nc, tc, emb8, hh8, hhT, qwT, qb, w2, b2, src, out)
    nc.compile()
    return nc


def _emit(nc, tc, emb8, hh8, hhT, qwT, qb, w2, b2, src, out):
    with (
        tc.tile_pool(name="persist", bufs=1) as pw,
        tc.tile_pool(name="small", bufs=2) as psm,
        tc.tile_pool(name="ps_m", bufs=3, space="PSUM") as ps_m,
        tc.tile_pool(name="ps_gen", bufs=2, space="PSUM") as ps_gen,
    ):
        # ---- persistent SBUF ----
        e_sb = pw.tile([P, V], F16)  # (t, v) exp(gen/32) - 62.5KB/partition
        hh_sb = pw.tile([P, KC, 2, P], F16)  # (d, kc, {tgt,src}, t/s)
        hh8_sb = pw.tile([P, G, 2, NT], F8)  # (d, g, i, t) DoubleRow weights
        qwT_sb = pw.tile([P, KC, D], F16)  # (d, kc, i)
        qkT_sb = pw.tile([P, KC, 2, P], F16)  # (i, ic, {q,k}, t/s)
        k_sb = pw.tile([P, D], F16)  # (s, i)
        xT_sb = pw.tile([P, D], F32)  # (i_p, (ic t))
        attn_sb = pw.tile([P, NS], F32)  # (t, s)
        attnT_sb = pw.tile([P, NT], F16)  # (s, t)
        a_sb = pw.tile([P, 1], F32)  # (t,)
        src_sb = pw.tile([P, 1], F32)  # (s,)
        w2_sb = pw.tile([P, KC], F32)
        b2_sb = pw.tile([1, 1], F32)
        qb_sb = pw.tile([1, D], F16)
        iota_sb = pw.tile([P, DCH], I16)
        iota16 = pw.tile([P, DCH], F16)  # exact: values < 2048
        identity = pw.tile([P, P], F32)
        ones16 = pw.tile([1, 2 * P], F16)
        ones32 = pw.tile([1, P], F32)
        zparts = pw.tile([P, NZ], F32)
        zcol = pw.tile([P, 1], F32)
        c1_sb = pw.tile([P, 1], F32)
        c2_sb = pw.tile([P, 1], F32)

        make_identity(nc, identity[:])
        nc.vector.memset(ones16[:], 1.0)
        nc.vector.memset(ones32[:], 1.0)
        nc.gpsimd.iota(iota_sb[:], pattern=[[1, DCH]], base=0, channel_multiplier=0)
        nc.vector.tensor_copy(out=iota16[:], in_=iota_sb[:])
        # gen matmul dep goes first on the SP queue; the emb stream follows it
        # immediately. Everything attention needs rides the Activation DGE
        # queue so it never delays the bandwidth-critical emb stream.
        nc.sync.dma_start(out=hh8_sb[:], in_=hh8.rearrange("g i p t -> p g i t"))
        nc.scalar.dma_start(out=src_sb[:], in_=src)
        nc.scalar.dma_start(out=qb_sb[:], in_=qb)
        nc.scalar.dma_start(out=b2_sb[:], in_=b2)
        nc.scalar.dma_start(out=w2_sb[:], in_=w2.rearrange("kc p -> p kc"))
        nc.scalar.dma_start(out=hh_sb[:], in_=hhT.rearrange("kc p w t -> p kc w t"))
        nc.scalar.dma_start(out=qwT_sb[:], in_=qwT.rearrange("kc p i -> p kc i"))

        def emit_attention():
            for ic in range(KC):
                qkT_ps = ps_m.tile([P, 2 * P], F32, tag="m")
                for kc in range(KC):
                    nc.tensor.matmul(
                        out=qkT_ps[:],
                        lhsT=qwT_sb[:, kc, ic * P : (ic + 1) * P],
                        rhs=hh_sb[:, kc],
                        start=(kc == 0),
                        stop=False,
                    )
                nc.tensor.matmul(
                    out=qkT_ps[:],
                    lhsT=qb_sb[:, ic * P : (ic + 1) * P],
                    rhs=ones16[:],
                    start=False,
                    stop=True,
                )
                nc.vector.tensor_copy(
                    out=qkT_sb[:, ic],
                    in_=qkT_ps[:].rearrange("i (w t) -> i w t", t=P),
                )

            k_ps = ps_m.tile([P, D], F32, tag="m")
            for kc in range(KC):
                nc.tensor.matmul(
                    out=k_ps[:],
                    lhsT=hh_sb[:, kc, 1, :],
                    rhs=qwT_sb[:, kc, :],
                    start=(kc == 0),
                    stop=False,
                )
            nc.tensor.matmul(
                out=k_ps[:],
                lhsT=ones16[:, 0:P],
                rhs=qb_sb[:],
                start=False,
                stop=True,
            )
            nc.vector.tensor_copy(out=k_sb[:], in_=k_ps[:])

            s_ps = ps_m.tile([P, P], F32, tag="m")
            for ic in range(KC):
                nc.tensor.matmul(
                    out=s_ps[:],
                    lhsT=qkT_sb[:, ic, 0, :],
                    rhs=qkT_sb[:, ic, 1, :],
                    start=(ic == 0),
                    stop=(ic == KC - 1),
                )
            m_col = psm.tile([P, 1], F32, tag="m")
            negm = psm.tile([P, 1], F32, tag="negm")
            zatt = psm.tile([P, 1], F32, tag="zatt")
            rz = psm.tile([P, 1], F32, tag="rz")
            nc.vector.reduce_max(
                out=m_col[:], in_=s_ps[:], axis=mybir.AxisListType.X
            )
            nc.vector.tensor_scalar_mul(negm[:], m_col[:], -INV_SQRT_D)
            nc.scalar.activation(
                out=attn_sb[:],
                in_=s_ps[:],
                func=AF.Exp,
                bias=negm[:],
                scale=INV_SQRT_D,
                accum_out=zatt[:],
            )
            nc.vector.reciprocal(rz[:], zatt[:])
            nc.vector.tensor_scalar_mul(attn_sb[:], attn_sb[:], rz[:])

            t_ps = ps_m.tile([P, P], F32, tag="m")
            nc.tensor.transpose(t_ps[:], attn_sb[:], identity[:])
            nc.vector.tensor_copy(out=attnT_sb[:], in_=t_ps[:])

            x_ps = ps_m.tile([P, D], F32, tag="m")
            for ic in range(KC):
                nc.tensor.matmul(
                    out=x_ps[:, ic * P : (ic + 1) * P],
                    lhsT=k_sb[:, ic * P : (ic + 1) * P],
                    rhs=attnT_sb[:],
                    start=True,
                    stop=True,
                )
            nc.vector.tensor_copy(out=xT_sb[:], in_=x_ps[:])

            c_ps = ps_m.tile([P, 1], F32, tag="m")
            for ic in range(KC):
                nc.tensor.matmul(
                    out=c_ps[:],
                    lhsT=xT_sb[:, ic * P : (ic + 1) * P],
                    rhs=w2_sb[:, ic : ic + 1],
                    start=(ic == 0),
                    stop=False,
                )
            nc.tensor.matmul(
                out=c_ps[:],
                lhsT=ones32[:],
                rhs=b2_sb[:],
                start=False,
                stop=True,
            )
            nc.scalar.activation(out=a_sb[:], in_=c_ps[:], func=AF.Sigmoid)

        # ---- pass 1: e = exp((htgt @ embT)/32), fp8 DoubleRow, streamed ----
        # first two loads are half-width so the PE starts sooner
        blocks = [(0, WCH), (WCH, WCH)] + [
            (nd * DCH, DCH) for nd in range(1, NDMA)
        ]
        with tc.tile_pool(name="embst", bufs=4) as pemb:
            for bi, (v0, width) in enumerate(blocks):
                emb_t = pemb.tile([P, G, 2, DCH], F8, tag="emb")
                nc.sync.dma_start(
                    out=emb_t[:, :, :, 0:width],
                    in_=emb8.rearrange("g i p v -> p g i v")[
                        :, :, :, v0 : v0 + width
                    ],
                )
                for w in range(width // WCH):
                    # bank-aligned halves: each matmul group stays in one
                    # 512-f32 PSUM bank; cols 500:512 are never written/read
                    g_ps = ps_gen.tile([P, 2, 512], F32, tag="g")
                    for h in range(2):
                        c0 = w * WCH + h * CH
                        for g in range(G):
                            nc.tensor.matmul(
                                out=g_ps[:, h, 0:CH],
                                lhsT=hh8_sb[:, g],
                                rhs=emb_t[:, g, :, c0 : c0 + CH],
                                perf_mode=DR,
                                start=(g == 0),
                                stop=(g == G - 1),
                            )
                    nz = (v0 + w * WCH) // WCH
                    nc.scalar.activation(
                        out=e_sb[:, nz * WCH : (nz + 1) * WCH].rearrange(
                            "t (h c) -> t h c", c=CH
                        ),
                        in_=g_ps[:, :, 0:CH],
                        func=AF.Exp,
                        scale=1.0 / ESCALE,
                        accum_out=zparts[:, nz : nz + 1],
                    )
                if bi == 0:
                    emit_attention()

        # ---- Z, c1 = (1-a)/Z, c2 = a*Z/(1-a) ----
        oma = psm.tile([P, 1], F32, tag="oma")
        roma = psm.tile([P, 1], F32, tag="roma")
        rzg = psm.tile([P, 1], F32, tag="rzg")
        az = psm.tile([P, 1], F32, tag="az")
        nc.vector.reduce_sum(out=zcol[:], in_=zparts[:], axis=mybir.AxisListType.X)
        nc.vector.tensor_scalar(
            out=oma[:],
            in0=a_sb[:],
            scalar1=-1.0,
            scalar2=1.0,
            op0=ALU.mult,
            op1=ALU.add,
        )
        nc.vector.reciprocal(roma[:], oma[:])
        nc.vector.reciprocal(rzg[:], zcol[:])
        nc.vector.tensor_tensor(out=c1_sb[:], in0=oma[:], in1=rzg[:], op=ALU.mult)
        nc.vector.tensor_tensor(out=az[:], in0=a_sb[:], in1=zcol[:], op=ALU.mult)
        nc.vector.tensor_tensor(out=c2_sb[:], in0=az[:], in1=roma[:], op=ALU.mult)

        # ---- pass 2: copy scatter + blend + log + store ----
        # out staging at 2*DCH (4000 cols): 8 output DMAs of 2MB, wide Ln
        OCH = 2 * DCH
        with tc.tile_pool(name="p2", bufs=2) as p2:
            for no in range(V // OCH):
                blend = p2.tile([P, OCH], F32, tag="blend")
                for oh2 in range(OCH // DCH):
                    nd = no * (OCH // DCH) + oh2
                    srcsh = p2.tile([P, 1], F32, tag="srcsh")
                    nc.vector.tensor_scalar(
                        out=srcsh[:],
                        in0=src_sb[:],
                        scalar1=1.0,
                        scalar2=-float(nd * DCH),
                        op0=ALU.mult,
                        op1=ALU.add,
                    )
                    onehot = p2.tile([P, DCH], F16, tag="oh")
                    nc.vector.tensor_scalar(
                        out=onehot[:],
                        in0=iota16[:],
                        scalar1=srcsh[:],
                        scalar2=None,
                        op0=ALU.is_equal,
                    )
                    for sc in range(DCH // CH):
                        n = nd * (DCH // CH) + sc
                        cp_ps = ps_m.tile([P, CH], F32, tag="m")
                        nc.tensor.matmul(
                            out=cp_ps[:],
                            lhsT=attnT_sb[:],
                            rhs=onehot[:, sc * CH : (sc + 1) * CH],
                            start=True,
                            stop=True,
                        )
                        nc.vector.scalar_tensor_tensor(
                            out=blend[
                                :, (oh2 * DCH + sc * CH) : (oh2 * DCH + (sc + 1) * CH)
                            ],
                            in0=cp_ps[:],
                            scalar=c2_sb[:],
                            in1=e_sb[:, n * CH : (n + 1) * CH],
                            op0=ALU.mult,
                            op1=ALU.add,
                        )
                outt = p2.tile([P, OCH], F32, tag="outt")
                nc.scalar.activation(
                    out=outt[:],
                    in_=blend[:],
                    func=AF.Ln,
                    scale=c1_sb[:],
                )
                nc.sync.dma_start(
                    out=out[:, no * OCH : (no + 1) * OCH], in_=outt[:]
                )


_NC_CACHE = []


def _get_nc():
    if not _NC_CACHE:
        _NC_CACHE.append(build_kernel())
    return _NC_CACHE[0]


def _f8(x):
    return x.astype(mybir.dt.np(F8))


def _make_in_maps(inputs):
    htgt = np.asarray(inputs["htgt"], dtype=np.float32)
    hsrc = np.asarray(inputs["hsrc"], dtype=np.float32)
    src = np.asarray(inputs["src"]).astype(np.float32)  # exact for v < 2^24
    emb = np.asarray(inputs["emb_weight"], dtype=np.float32)
    q_w = np.asarray(inputs["q_w"], dtype=np.float32)
    q_b = np.asarray(inputs["q_b"], dtype=np.float32)
    f_w = np.asarray(inputs["f_w"], dtype=np.float32)
    f_b = np.asarray(inputs["f_b"], dtype=np.float32)
    copy_w = np.asarray(inputs["copy_w"], dtype=np.float32)
    copy_b = np.asarray(inputs["copy_b"], dtype=np.float32)

    # (G, 2, P, V): d = g*256 + i*128 + p, scaled into fp8e4 normal range
    emb8 = np.ascontiguousarray(_f8((emb.T * ESCALE).reshape(G, 2, P, V)))
    qwT = np.ascontiguousarray(q_w.T.astype(np.float16).reshape(KC, P, D))
    qb = np.ascontiguousarray(q_b.astype(np.float16).reshape(1, D))
    w2 = np.ascontiguousarray((f_w.T @ copy_w[0]).astype(np.float32).reshape(KC, P))
    b2 = np.ascontiguousarray(
        (copy_w[0] @ f_b + copy_b[0]).astype(np.float32).reshape(1, 1)
    )

    in_maps = []
    for c in range(NCORES):
        hh = np.stack([htgt[:, c, :].T, hsrc[:, c, :].T], axis=1)  # (D, 2, P)
        hhT = np.ascontiguousarray(hh.astype(np.float16).reshape(KC, P, 2, P))
        hh8 = np.ascontiguousarray(_f8(htgt[:, c, :].T.reshape(G, 2, P, NT)))
        in_maps.append(
            {
                "emb8": emb8,
                "hh8": hh8,
                "hhT": hhT,
                "qwT": qwT,
                "qb": qb,
                "w2": w2,
                "b2": b2,
                "src": np.ascontiguousarray(src[:, c].reshape(NS, 1)),
            }
        )
    return in_maps


def kernel(**inputs):
    in_maps = _make_in_maps(inputs)
    nc = _get_nc()
    res = run_bass_kernel_spmd(nc, in_maps, list(range(NCORES))).results
    return np.stack([res[c]["out"] for c in range(NCORES)], axis=1)


# revision 22
# speedup vs baseline: 1.1576x; 1.1576x over previous
"""CopyGenerator kernel for Trainium2 (Bass/Tile), batch-parallel over 8 cores.

Core c owns batch c end-to-end: attention, full-vocab generation scores,
softmax normalizer Z, copy-distribution scatter, and the blended log output.
No collectives — each core's NEFF is fully independent, so per-core exec
time is its own busy span regardless of dispatch skew across cores.

Host-side prep (numpy, once per call):
  emb8  = (emb.T * 32) cast to fp8e4, [KC/2, 2, P, V]  (DoubleRow k-pairs)
  hh8   = htgt[:,c,:].T cast fp8e4 for the gen matmul
  hhT   = htgt/hsrc own-batch transposes fp16 (attention)
  qwT   = q_w.T fp16;  w2 = f_w.T @ copy_w.T, b2 = copy_w @ f_b + copy_b

Device pipeline per core (vocab in 2000-col blocks; first two half-width
so the PE starts sooner; attention emitted after the first gen block):
  pass 1: DMA emb8 block -> gen matmul (PE fp8 DoubleRow: 2 instrs per
          500-col x 512-deep) -> exp(score/32) over 1000 cols (ACT, fused
          row-sum) -> e[:, block] SBUF-resident ([128, 32000] f16)
  Z = rowsum; c1 = (1-a)/Z, c2 = a*Z/(1-a)
  pass 2: onehot(src) per 2000 cols (DVE is_equal vs f16 iota, 2x mode) ->
          copy matmul (PE fp16, 500-col) -> blend c2*cp+e (DVE) ->
          Ln with scale c1 over 2000 cols (ACT) -> DMA out rows
"""

import sys

sys.path.insert(0, "/opt/trn_rl_repo")

import numpy as np

from concourse import bass, bacc, mybir
import concourse.tile as tile
from concourse.bass_utils import run_bass_kernel_spmd
from concourse.masks import make_identity

NT, NS, BS, D, V = 128, 128, 8, 512, 32000
NCORES = 8
P = 128
KC = D // P  # 4 contraction chunks of 128
G = KC // 2  # 2 DoubleRow pair-groups (256-deep each)
CH = 500  # cols per PSUM bank (f32)
WCH = 2 * CH  # cols per exp activate (2 banks)
DCH = 2000  # cols per embT DMA load / onehot / Ln / out store
NDMA = V // DCH  # 16
NZ = V // WCH  # 32 partial-Z columns
ESCALE = 32.0  # host scales emb by 32 into fp8e4 normal range; exp undoes
F32 = mybir.dt.float32
F16 = mybir.dt.float16
F8 = mybir.dt.float8e4
I16 = mybir.dt.int16
AF = mybir.ActivationFunctionType
ALU = mybir.AluOpType
DR = mybir.MatmulPerfMode.DoubleRow
INV_SQRT_D = 1.0 / float(np.sqrt(np.float32(D)))


def build_kernel():
    nc = bacc.Bacc(
        "TRN2",
        target_bir_lowering=False,
        debug=False,
        enable_asserts=False,
        num_devices=NCORES,
    )
    emb8 = nc.dram_tensor("emb8", [G, 2, P, V], F8, kind="ExternalInput").ap()
    hh8 = nc.dram_tensor("hh8", [G, 2, P, NT], F8, kind="ExternalInput").ap()
    hhT = nc.dram_tensor("hhT", [KC, P, 2, P], F16, kind="ExternalInput").ap()
    qwT = nc.dram_tensor("qwT", [KC, P, D], F16, kind="ExternalInput").ap()
    qb = nc.dram_tensor("qb", [1, D], F16, kind="ExternalInput").ap()
    w2 = nc.dram_tensor("w2", [KC, P], F32, kind="ExternalInput").ap()
    b2 = nc.dram_tensor("b2", [1, 1], F32, kind="ExternalInput").ap()
    src = nc.dram_tensor("src", [NS, 1], F32, kind="ExternalInput").ap()
    out = nc.dram_tensor("out", [NT, V], F32, kind="ExternalOutput").ap()

    with tile.TileContext(nc) as tc:
        _emit(nc, tc, emb8, hh8, hhT, qwT, qb, w2, b2, src, out)
    nc.compile()
    return nc


def _emit(nc, tc, emb8, hh8, hhT, qwT, qb, w2, b2, src, out):
    with (
        tc.tile_pool(name="persist", bufs=1) as pw,
        tc.tile_pool(name="small", bufs=2) as psm,
        tc.tile_pool(name="ps_m", bufs=3, space="PSUM") as ps_m,
        tc.tile_pool(name="ps_gen", bufs=2, space="PSUM") as ps_gen,
    ):
        # ---- persistent SBUF ----
        e_sb = pw.tile([P, V], F16)  # (t, v) exp(gen/32) - 62.5KB/partition
        hh_sb = pw.tile([P, KC, 2, P], F16)  # (d, kc, {tgt,src}, t/s)
        hh8_sb = pw.tile([P, G, 2, NT], F8)  # (d, g, i, t) DoubleRow weights
        qwT_sb = pw.tile([P, KC, D], F16)  # (d, kc, i)
        qkT_sb = pw.tile([P, KC, 2, P], F16)  # (i, ic, {q,k}, t/s)
        k_sb = pw.tile([P, D], F16)  # (s, i)
        xT_sb = pw.tile([P, D], F32)  # (i_p, (ic t))
        attn_sb = pw.tile([P, NS], F32)  # (t, s)
        attnT_sb = pw.tile([P, NT], F16)  # (s, t)
        a_sb = pw.tile([P, 1], F32)  # (t,)
        src_sb = pw.tile([P, 1], F32)  # (s,)
        w2_sb = pw.tile([P, KC], F32)
        b2_sb = pw.tile([1, 1], F32)
        qb_sb = pw.tile([1, D], F16)
        iota_sb = pw.tile([P, DCH], I16)
        iota16 = pw.tile([P, DCH], F16)  # exact: values < 2048
        identity = pw.tile([P, P], F32)
        ones16 = pw.tile([1, 2 * P], F16)
        ones32 = pw.tile([1, P], F32)
        zparts = pw.tile([P, NZ], F32)
        zcol = pw.tile([P, 1], F32)
        c1_sb = pw.tile([P, 1], F32)
        c2_sb = pw.tile([P, 1], F32)

        make_identity(nc, identity[:])
        nc.vector.memset(ones16[:], 1.0)
        nc.vector.memset(ones32[:], 1.0)
        nc.gpsimd.iota(iota_sb[:], pattern=[[1, DCH]], base=0, channel_multiplier=0)
        nc.vector.tensor_copy(out=iota16[:], in_=iota_sb[:])
        # gen matmul dep goes first so the PE can start asap; the rest of the
        # preamble is small (~0.77MB) and rides ahead of the emb stream.
        nc.sync.dma_start(out=hh8_sb[:], in_=hh8.rearrange("g i p t -> p g i t"))
        nc.sync.dma_start(out=src_sb[:], in_=src)
        nc.sync.dma_start(out=qb_sb[:], in_=qb)
        nc.sync.dma_start(out=b2_sb[:], in_=b2)
        nc.sync.dma_start(out=w2_sb[:], in_=w2.rearrange("kc p -> p kc"))
        nc.sync.dma_start(out=hh_sb[:], in_=hhT.rearrange("kc p w t -> p kc w t"))
        nc.sync.dma_start(out=qwT_sb[:], in_=qwT.rearrange("kc p i -> p kc i"))

        def emit_attention():
            for ic in range(KC):
                qkT_ps = ps_m.tile([P, 2 * P], F32, tag="m")
                for kc in range(KC):
                    nc.tensor.matmul(
                        out=qkT_ps[:],
                        lhsT=qwT_sb[:, kc, ic * P : (ic + 1) * P],
                        rhs=hh_sb[:, kc],
                        start=(kc == 0),
                        stop=False,
                    )
                nc.tensor.matmul(
                    out=qkT_ps[:],
                    lhsT=qb_sb[:, ic * P : (ic + 1) * P],
                    rhs=ones16[:],
                    start=False,
                    stop=True,
                )
                nc.vector.tensor_copy(
                    out=qkT_sb[:, ic],
                    in_=qkT_ps[:].rearrange("i (w t) -> i w t", t=P),
                )

            k_ps = ps_m.tile([P, D], F32, tag="m")
            for kc in range(KC):
                nc.tensor.matmul(
                    out=k_ps[:],
                    lhsT=hh_sb[:, kc, 1, :],
                    rhs=qwT_sb[:, kc, :],
                    start=(kc == 0),
                    stop=False,
                )
            nc.tensor.matmul(
                out=k_ps[:],
                lhsT=ones16[:, 0:P],
                rhs=qb_sb[:],
                start=False,
                stop=True,
            )
            nc.vector.tensor_copy(out=k_sb[:], in_=k_ps[:])

            s_ps = ps_m.tile([P, P], F32, tag="m")
            for ic in range(KC):
                nc.tensor.matmul(
                    out=s_ps[:],
                    lhsT=qkT_sb[:, ic, 0, :],
                    rhs=qkT_sb[:, ic, 1, :],
                    start=(ic == 0),
                    stop=(ic == KC - 1),
                )
            m_col = psm.tile([P, 1], F32, tag="m")
            negm = psm.tile([P, 1], F32, tag="negm")
            zatt = psm.tile([P, 1], F32, tag="zatt")
            rz = psm.tile([P, 1], F32, tag="rz")
            nc.vector.reduce_max(
                out=m_col[:], in_=s_ps[:], axis=mybir.AxisListType.X
            )
            nc.vector.tensor_scalar_mul(negm[:], m_col[:], -INV_SQRT_D)
            nc.scalar.activation(
                out=attn_sb[:],
                in_=s_ps[:],
                func=AF.Exp,
                bias=negm[:],
                scale=INV_SQRT_D,
                accum_out=zatt[:],
            )
            nc.vector.reciprocal(rz[:], zatt[:])
            nc.vector.tensor_scalar_mul(attn_sb[:], attn_sb[:], rz[:])

            t_ps = ps_m.tile([P, P], F32, tag="m")
            nc.tensor.transpose(t_ps[:], attn_sb[:], identity[:])
            nc.vector.tensor_copy(out=attnT_sb[:], in_=t_ps[:])

            x_ps = ps_m.tile([P, D], F32, tag="m")
            for ic in range(KC):
                nc.tensor.matmul(
                    out=x_ps[:, ic * P : (ic + 1) * P],
                    lhsT=k_sb[:, ic * P : (ic + 1) * P],
                    rhs=attnT_sb[:],
                    start=True,
                    stop=True,
                )
            nc.vector.tensor_copy(out=xT_sb[:], in_=x_ps[:])

            c_ps = ps_m.tile([P, 1], F32, tag="m")
            for ic in range(KC):
                nc.tensor.matmul(
                    out=c_ps[:],
                    lhsT=xT_sb[:, ic * P : (ic + 1) * P],
                    rhs=w2_sb[:, ic : ic + 1],
                    start=(ic == 0),
                    stop=False,
                )
            nc.tensor.matmul(
                out=c_ps[:],
                lhsT=ones32[:],
                rhs=b2_sb[:],
                start=False,
                stop=True,
            )
            nc.scalar.activation(out=a_sb[:], in_=c_ps[:], func=AF.Sigmoid)

        # ---- pass 1: e = exp((htgt @ embT)/32), fp8 DoubleRow, streamed ----
        # first two loads are half-width so the PE starts sooner
        blocks = [(0, WCH), (WCH, WCH)] + [
            (nd * DCH, DCH) for nd in range(1, NDMA)
        ]
        with tc.tile_pool(name="embst", bufs=4) as pemb:
            for bi, (v0, width) in enumerate(blocks):
                emb_t = pemb.tile([P, G, 2, DCH], F8, tag="emb")
                nc.sync.dma_start(
                    out=emb_t[:, :, :, 0:width],
                    in_=emb8.rearrange("g i p v -> p g i v")[
                        :, :, :, v0 : v0 + width
                    ],
                )
                for w in range(width // WCH):
                    # bank-aligned halves: each matmul group stays in one
                    # 512-f32 PSUM bank; cols 500:512 are never written/read
                    g_ps = ps_gen.tile([P, 2, 512], F32, tag="g")
                    for h in range(2):
                        c0 = w * WCH + h * CH
                        for g in range(G):
                            nc.tensor.matmul(
                                out=g_ps[:, h, 0:CH],
                                lhsT=hh8_sb[:, g],
                                rhs=emb_t[:, g, :, c0 : c0 + CH],
                                perf_mode=DR,
                                start=(g == 0),
                                stop=(g == G - 1),
                            )
                    nz = (v0 + w * WCH) // WCH
                    nc.scalar.activation(
                        out=e_sb[:, nz * WCH : (nz + 1) * WCH].rearrange(
                            "t (h c) -> t h c", c=CH
                        ),
                        in_=g_ps[:, :, 0:CH],
                        func=AF.Exp,
                        scale=1.0 / ESCALE,
                        accum_out=zparts[:, nz : nz + 1],
                    )
                if bi == 3:
                    emit_attention()

        # ---- Z, c1 = (1-a)/Z, c2 = a*Z/(1-a) ----
        oma = psm.tile([P, 1], F32, tag="oma")
        roma = psm.tile([P, 1], F32, tag="roma")
        rzg = psm.tile([P, 1], F32, tag="rzg")
        az = psm.tile([P, 1], F32, tag="az")
        nc.vector.reduce_sum(out=zcol[:], in_=zparts[:], axis=mybir.AxisListType.X)
        nc.vector.tensor_scalar(
            out=oma[:],
            in0=a_sb[:],
            scalar1=-1.0,
            scalar2=1.0,
            op0=ALU.mult,
            op1=ALU.add,
        )
        nc.vector.reciprocal(roma[:], oma[:])
        nc.vector.reciprocal(rzg[:], zcol[:])
        nc.vector.tensor_tensor(out=c1_sb[:], in0=oma[:], in1=rzg[:], op=ALU.mult)
        nc.vector.tensor_tensor(out=az[:], in0=a_sb[:], in1=zcol[:], op=ALU.mult)
        nc.vector.tensor_tensor(out=c2_sb[:], in0=az[:], in1=roma[:], op=ALU.mult)

        # ---- pass 2: copy scatter + blend + log + store ----
        # out staging at 2*DCH (4000 cols): 8 output DMAs of 2MB, wide Ln
        OCH = 2 * DCH
        with tc.tile_pool(name="p2", bufs=2) as p2:
            for no in range(V // OCH):
                blend = p2.tile([P, OCH], F32, tag="blend")
                for oh2 in range(OCH // DCH):
                    nd = no * (OCH // DCH) + oh2
                    srcsh = p2.tile([P, 1], F32, tag="srcsh")
                    nc.vector.tensor_scalar(
                        out=srcsh[:],
                        in0=src_sb[:],
                        scalar1=1.0,
                        scalar2=-float(nd * DCH),
                        op0=ALU.mult,
                        op1=ALU.add,
                    )
                    onehot = p2.tile([P, DCH], F16, tag="oh")
                    nc.vector.tensor_scalar(
                        out=onehot[:],
                        in0=iota16[:],
                        scalar1=srcsh[:],
                        scalar2=None,
                        op0=ALU.is_equal,
                    )
                    for sc in range(DCH // CH):
                        n = nd * (DCH // CH) + sc
                        cp_ps = ps_m.tile([P, CH], F32, tag="m")
                        nc.tensor.matmul(
                            out=cp_ps[:],
                            lhsT=attnT_sb[:],
                            rhs=onehot[:, sc * CH : (sc + 1) * CH],
                            start=True,
                            stop=True,
                        )
                        nc.vector.scalar_tensor_tensor(
                            out=blend[
                                :, (oh2 * DCH + sc * CH) : (oh2 * DCH + (sc + 1) * CH)
                            ],
                            in0=cp_ps[:],
                            scalar=c2_sb[:],
                            in1=e_sb[:, n * CH : (n + 1) * CH],
                            op0=ALU.mult,
                            op1=ALU.add,
                        )
                outt = p2.tile([P, OCH], F32, tag="outt")
                nc.scalar.activation(
                    out=outt[:],
                    in_=blend[:],
                    func=AF.Ln,
                    scale=c1_sb[:],
                )
                nc.sync.dma_start(
                    out=out[:, no * OCH : (no + 1) * OCH], in_=outt[:]
                )


_NC_CACHE = []


def _get_nc():
    if not _NC_CACHE:
        _NC_CACHE.append(build_kernel())
    return _NC_CACHE[0]


def _f8(x):
    return x.astype(mybir.dt.np(F8))


def _make_in_maps(inputs):
    htgt = np.asarray(inputs["htgt"], dtype=np.float32)
    hsrc = np.asarray(inputs["hsrc"], dtype=np.float32)
    src = np.asarray(inputs["src"]).astype(np.float32)  # exact for v < 2^24
    emb = np.asarray(inputs["emb_weight"], dtype=np.float32)
    q_w = np.asarray(inputs["q_w"], dtype=np.float32)
    q_b = np.asarray(inputs["q_b"], dtype=np.float32)
    f_w = np.asarray(inputs["f_w"], dtype=np.float32)
    f_b = np.asarray(inputs["f_b"], dtype=np.float32)
    copy_w = np.asarray(inputs["copy_w"], dtype=np.float32)
    copy_b = np.asarray(inputs["copy_b"], dtype=np.float32)

    # (G, 2, P, V): d = g*256 + i*128 + p, scaled into fp8e4 normal range
    emb8 = np.ascontiguousarray(_f8((emb.T * ESCALE).reshape(G, 2, P, V)))
    qwT = np.ascontiguousarray(q_w.T.astype(np.float16).reshape(KC, P, D))
    qb = np.ascontiguousarray(q_b.astype(np.float16).reshape(1, D))
    w2 = np.ascontiguousarray((f_w.T @ copy_w[0]).astype(np.float32).reshape(KC, P))
    b2 = np.ascontiguousarray(
        (copy_w[0] @ f_b + copy_b[0]).astype(np.float32).reshape(1, 1)
    )

    in_maps = []
    for c in range(NCORES):
        hh = np.stack([htgt[:, c, :].T, hsrc[:, c, :].T], axis=1)  # (D, 2, P)
        hhT = np.ascontiguousarray(hh.astype(np.float16).reshape(KC, P, 2, P))
        hh8 = np.ascontiguousarray(_f8(htgt[:, c, :].T.reshape(G, 2, P, NT)))
        in_maps.append(
            {
                "emb8": emb8,
                "hh8": hh8,
                "hhT": hhT,
                "qwT": qwT,
                "qb": qb,
                "w2": w2,
                "b2": b2,
                "src": np.ascontiguousarray(src[:, c].reshape(NS, 1)),
            }
        )
    return in_maps


def kernel(**inputs):
    in_maps = _make_in_maps(inputs)
    nc = _get_nc()
    res = run_bass_kernel_spmd(nc, in_maps, list(range(NCORES))).results
    return np.stack([res[c]["out"] for c in range(NCORES)], axis=1)
